# revision 5
# baseline (speedup 1.0000x reference)
"""BayesLinear sampling kernel for 8 Trainium2 NeuronCores — v4.

Computes out[n,o] = sum_i x[n,i]*(mu_w[i,o] + sigma_w[i,o]*eps_w[n,i,o])
                    + mu_b[o] + sigma_b[o]*eps_b[n,o]
with N=4096, IN=OUT=256, data-parallel over the sample dim N (512
samples per core).  The dominant cost is streaming eps_w (1 GiB fp32)
from HBM once; per-core DMA roofline ~375 us at ~358 GB/s.

Design: 32-sample-group PSUM accumulation + a FLAT eps DMA layout:
each 1 MiB DMA reads 4 consecutive samples' [256,256] blocks as one
fully-sequential DRAM stream (8 KiB contiguous per partition, vs v1's
scattered 1 KiB chunks).  Partition p of an eps tile then holds i-rows
[(p%32)*8, (p%32)*8+8) of sample p//32.  Matmul (t, j) of a group
contracts over all 128 partitions with a block-diagonal stationary
window [128,32] whose only nonzero columns are [4t, 4t+4) (sample
t*4+m of the 4-sample tile lives on partitions [32m, 32m+32)), so 64
matmuls of [K=128, M=32, N=256] accumulate a whole 32-sample group in
one PSUM tile -- extraction stays a single partition-aligned
[32, 256] scalar copy per group.  Stationary windows live packed in a
[128, 2304] tile: window (t,j) starts at col 288j+32t and its nonzero
block sits at cols 36*(8j+t)+[0,4), so blocks never fall inside other
windows; a group refresh is 4 strided DVE copies (one per m) from a
host-packed dense [128, 16*256] xstat table.  The sigma tile is
identical for every eps tile.  The sigma multiply runs IN PLACE on the
eps tile through its float32r view (the rounded-producer for the fp32r
matmul moving operand), halving the eps pool footprint so 10 DMA
buffers keep the HBM stream deep.  Interleaved head-to-head slope
timing vs the previous kernel measured ~400 us vs ~430 us per
iteration (per-core roofline ~375 us for the 134 MiB eps stream).
"""

import sys
from contextlib import nullcontext

if "/opt/trn_rl_repo" not in sys.path:
    sys.path.insert(0, "/opt/trn_rl_repo")

import numpy as np

import concourse.bacc as bacc
import concourse.mybir as mybir
from concourse.bass_utils import run_bass_kernel_spmd
from concourse.tile import TileContext

N, IN, OUT = 4096, 256, 256
N_CORES = 8
B = N // N_CORES  # samples per core (512)
F32 = mybir.dt.float32
F32R = mybir.dt.float32r

# knobs
ST = 4          # samples per eps DMA tile (1 MiB per DMA)
EPS_BUFS = 10   # in-flight eps tiles (1 MiB each; in-place sigma mul)
G_BUFS = 4      # rotating PSUM group accumulators ([32,256], 1 bank each)
DIAG_SETS = 4   # rotating sets of packed block-diag stationary tiles
IPP = IN // 32  # i-rows per partition (8)
DMW = 2304      # packed stationary tile width (window (t,j) at 288j+32t)

_CACHED = {}


def _build_nc(reps: int = 1, skip: tuple = ()):
    """Build the per-core bass program.  reps>1 wraps the main body in a
    Tile For_i loop that re-runs it on the same data -- used only by the
    timing harness (slope timing to cancel host/axon dispatch overhead)."""
    nc = bacc.Bacc("TRN2", target_bir_lowering=False, debug=False,
                   num_devices=N_CORES)

    n_tiles = B // ST           # eps tiles per core (128)
    n_groups = B // 32          # 32-sample groups per core (16)
    # same bytes as the [B, IN, OUT] shard: tile T = samples [T*ST, (T+1)*ST),
    # partition p = sample p//32, i-rows [(p%32)*IPP, (p%32)*IPP+IPP)
    eps_w = nc.declare_dram_parameter("eps_w", [n_tiles, 128, IPP * OUT], F32R,
                                      isOutput=False)
    xT = nc.declare_dram_parameter("xT", [IN, B], F32, isOutput=False)
    # xstat[p, g*256 + (j*8+t)*4 + m] = x[g*32+t*4+m, (p%32)*8+j] if p//32==m
    xstat = nc.declare_dram_parameter("xstat", [128, n_groups * 256], F32,
                                      isOutput=False)
    eps_b = nc.declare_dram_parameter("eps_b", [B, OUT], F32, isOutput=False)
    sigp = nc.declare_dram_parameter("sigp", [128, IPP * OUT], F32, isOutput=False)
    mup = nc.declare_dram_parameter("mup", [128, 2 * OUT], F32, isOutput=False)
    sgb_bc = nc.declare_dram_parameter("sgb_bc", [128, OUT], F32, isOutput=False)
    b1_bc = nc.declare_dram_parameter("b1_bc", [128, OUT], F32, isOutput=False)
    out = nc.declare_dram_parameter("out", [B, OUT], F32, isOutput=True)

    n_blk = B // 128            # 4 sample blocks per core

    with TileContext(nc) as tc:
        with (
            tc.tile_pool(name="const", bufs=1) as cpool,
            tc.tile_pool(name="psum", bufs=1, space="PSUM") as ppool,
            tc.tile_pool(name="eps", bufs=EPS_BUFS) as epool,
            tc.tile_pool(name="out", bufs=2) as opool,
        ):
            # --- constants (outside the timing loop) ---
            xt = [cpool.tile([128, B], F32, tag=f"xt{ic}", name=f"xt{ic}")
                  for ic in range(2)]
            for ic in range(2):
                nc.sync.dma_start(out=xt[ic][:, :], in_=xT[ic * 128:(ic + 1) * 128, :])
            xs = cpool.tile([128, n_groups * 256], F32, tag="xs")
            nc.sync.dma_start(out=xs[:, :], in_=xstat[:, :])
            sg = cpool.tile([128, IPP * OUT], F32, tag="sg")
            nc.sync.dma_start(out=sg[:, :], in_=sigp[:, :])
            mp = cpool.tile([128, 2 * OUT], F32, tag="mp")
            nc.sync.dma_start(out=mp[:, :], in_=mup[:, :])
            sgb = cpool.tile([128, OUT], F32, tag="sgb")
            nc.sync.dma_start(out=sgb[:, :], in_=sgb_bc[:, :])
            b1t = cpool.tile([128, OUT], F32, tag="b1t")
            nc.sync.dma_start(out=b1t[:, :], in_=b1_bc[:, :])
            # fp32r-rounded copies for matmul operands
            xtr = [cpool.tile([128, B], F32R, tag=f"xtr{ic}", name=f"xtr{ic}")
                   for ic in range(2)]
            for ic in range(2):
                nc.vector.tensor_copy(out=xtr[ic][:, :], in_=xt[ic][:, :])
            mpr = cpool.tile([128, 2 * OUT], F32R, tag="mpr")
            nc.vector.tensor_copy(out=mpr[:, :], in_=mp[:, :])
            # persistent packed block-diag stationaries: only the 4-col blocks
            # at 36k+m are ever rewritten, the zeros persist.  zero them via a
            # rounded DVE copy from a zero tile.
            zt = cpool.tile([128, DMW], F32, tag="zt")
            nc.vector.memset(zt[:, :], 0.0)
            dmask = []
            for ds in range(DIAG_SETS):
                dm = cpool.tile([128, DMW], F32R, tag=f"dm{ds}", name=f"dm{ds}")
                nc.vector.tensor_copy(out=dm[:, :], in_=zt[:, :])
                dmask.append(dm)

            loop = tc.For_i(0, reps, 1) if reps > 1 else nullcontext()
            with loop:
                for blk in range(n_blk):
                    bsl = slice(blk * 128, (blk + 1) * 128)
                    o_blk = opool.tile([128, OUT], F32, tag="o_blk")
                    eb = opool.tile([128, OUT], F32, tag="eb")
                    nc.sync.dma_start(out=eb[:, :], in_=eps_b[bsl, :])

                    for grp in range(4):  # 32-sample groups in this block
                        g = blk * 4 + grp           # global group index
                        g0 = g * 32                 # first sample of group
                        gl = grp * 32               # group base row in block
                        dm = dmask[g % DIAG_SETS]
                        # refresh the 4-col blocks: dm[:, 36k+m] <- xstat col
                        # g*256 + 4k + m, k = 8j+t = 0..63
                        for m in range(ST):
                            nc.vector.tensor_copy(
                                out=dm[:, m:m + 36 * 63 + 1:36],
                                in_=xs[:, g * 256 + m:g * 256 + m + 253:4],
                            )
                        g32 = ppool.tile([32, OUT], F32, tag="g32", bufs=G_BUFS,
                                         name="g32")
                        # mu term for the group (dense x-block stationary)
                        if "mu" not in skip:
                            for ic in range(2):
                                nc.tensor.matmul(
                                    g32[:, :],
                                    lhsT=xtr[ic][:, g0:g0 + 32],
                                    rhs=mpr[:, ic * OUT:(ic + 1) * OUT],
                                    start=(ic == 0),
                                    stop=False,
                                )
                        for t in range(8):  # eps tiles (4 samples each)
                            T = g * 8 + t   # global eps tile index
                            e = epool.tile([128, IPP * OUT], F32R, tag="e",
                                           name="e")
                            if "dma" not in skip:
                                nc.sync.dma_start(out=e[:, :], in_=eps_w[T])
                            if "dve" not in skip:
                                # in-place sigma multiply (rounded producer
                                # for the fp32r matmul moving operand)
                                nc.vector.tensor_mul(out=e[:, :], in0=e[:, :],
                                                     in1=sg[:, :])
                            if "mm" not in skip:
                                for j in range(IPP):
                                    last = (t == 7 and j == IPP - 1)
                                    nc.tensor.matmul(
                                        g32[:, :],
                                        lhsT=dm[:, 288 * j + 32 * t:
                                                288 * j + 32 * t + 32],
                                        rhs=e[:, j * OUT:(j + 1) * OUT],
                                        start=False,
                                        stop=last,
                                    )
                        if "ext" not in skip:
                            nc.scalar.copy(out=o_blk[gl:gl + 32, :],
                                           in_=g32[:, :])

                    # bias + writeback: out = o_blk + sgb*eps_b + b1
                    bt = opool.tile([128, OUT], F32, tag="bt")
                    nc.vector.tensor_mul(out=bt[:, :], in0=eb[:, :], in1=sgb[:, :])
                    nc.vector.tensor_add(out=bt[:, :], in0=bt[:, :], in1=b1t[:, :])
                    if "ext" not in skip:
                        nc.vector.tensor_add(out=bt[:, :], in0=bt[:, :],
                                             in1=o_blk[:, :])
                    nc.sync.dma_start(out=out[bsl, :], in_=bt[:, :])

    nc.compile()
    return nc


def _prep_in_maps(x, eps_w, eps_b, w_param1, logw_param2, b_param1, logb_param2):
    x = np.ascontiguousarray(np.asarray(x, dtype=np.float32))
    eps_w = np.asarray(eps_w, dtype=np.float32)
    eps_b = np.ascontiguousarray(np.asarray(eps_b, dtype=np.float32))
    w1 = np.asarray(w_param1, dtype=np.float32)
    lw2 = np.asarray(logw_param2, dtype=np.float32)
    b1 = np.asarray(b_param1, dtype=np.float32)
    lb2 = np.asarray(logb_param2, dtype=np.float32)

    n_tiles = B // ST
    n_groups = B // 32
    sigw = np.exp(lw2)  # [IN, OUT]
    # sg[p, j*OUT+o] = sigw[(p%32)*IPP + j, o]  (identical for every tile)
    sigp = np.ascontiguousarray(
        np.tile(sigw.reshape(32, IPP * OUT), (4, 1)))
    # mup[p, c*OUT + o] = w1[c*128+p, o]
    mup = np.ascontiguousarray(
        w1.reshape(2, 128, OUT).transpose(1, 0, 2).reshape(128, 2 * OUT)
    )
    sigb = np.exp(lb2)  # [OUT]
    sgb_bc = np.ascontiguousarray(np.broadcast_to(sigb, (128, OUT)))
    b1_bc = np.ascontiguousarray(np.broadcast_to(b1, (128, OUT)))

    in_maps = []
    for c in range(N_CORES):
        sl = slice(c * B, (c + 1) * B)
        xc = x[sl]                                  # [B, IN]
        # xstat[p, g*256 + (j*8+t)*4 + m] = xc[g*32+t*4+m, (p%32)*8+j]
        #   if p//32 == m else 0
        xg = xc.reshape(n_groups, 8, ST, 32, IPP)   # [g, t, m, a, j]
        xstat = np.zeros((ST, 32, n_groups, IPP, 8, ST), dtype=np.float32)
        for m in range(ST):
            # [g, t, a, j] -> [a, g, j, t]
            xstat[m, :, :, :, :, m] = xg[:, :, m].transpose(2, 0, 3, 1)
        xstat = np.ascontiguousarray(xstat.reshape(128, n_groups * 256))
        in_maps.append({
            "eps_w": np.ascontiguousarray(eps_w[sl]).reshape(
                n_tiles, 128, IPP * OUT),
            "xT": np.ascontiguousarray(xc.T),
            "xstat": xstat,
            "eps_b": np.ascontiguousarray(eps_b[sl]),
            "sigp": sigp,
            "mup": mup,
            "sgb_bc": sgb_bc,
            "b1_bc": b1_bc,
        })
    return in_maps


def kernel(x, eps_w, eps_b, w_param1, logw_param2, b_param1, logb_param2):
    if "nc" not in _CACHED:
        _CACHED["nc"] = _build_nc()
    nc = _CACHED["nc"]
    in_maps = _prep_in_maps(x, eps_w, eps_b, w_param1, logw_param2,
                            b_param1, logb_param2)
    res = run_bass_kernel_spmd(nc, in_maps, core_ids=list(range(N_CORES)))
    out = np.empty((N, OUT), dtype=np.float32)
    for c in range(N_CORES):
        out[c * B:(c + 1) * B] = res.results[c]["out"]
    return out
